# revision 8
# baseline (speedup 1.0000x reference)
"""Cross-attention (B=4, Sq=4096, Sk=1024, H=16, D=1024) on 8 TRN2 NeuronCores.

Sharding: tensor-parallel by heads. Core c owns heads (2c, 2c+1), i.e. columns
[128c, 128c+128) of Wq/Wk/Wv and rows [128c, 128c+128) of Wo.

Per-core dataflow (all activations kept feature-on-partition, "transposed"):
  qT[c,i] = sum_k Wq[k,c] xT[k,i]        (lhsT=Wq chunk, rhs=xT chunk)
  kT[c,j] likewise from yT; v[j,d] natural layout (lhsT=yT chunk, rhs=Wv chunk)
  scoresT[j,i] = kT_h[:,j].T @ qT_h[:,i]  (per head, row-packed across 2 heads)
  e = exp(scoresT)  (no max-subtraction: scores are O(1) by construction)
  noutT[d,i] (+ sums row via an appended ones column in v_aug) accumulated over j
  attT = noutT * (1/sums) broadcast (PE outer-product broadcast trick)
  AllToAll (per batch) head-shard -> seq-shard; out-proj on 512 rows/batch.

Host prep: x,y,W* are pre-transposed/pre-chunked and cast to bf16 on the host;
all matmuls run bf16 with fp32 PSUM accumulation; output is fp32.
"""

import numpy as np
import ml_dtypes

import concourse.bass as bass
import concourse.mybir as mybir
from concourse import bacc, tile
from concourse import bass_utils

BF16 = mybir.dt.bfloat16
F32 = mybir.dt.float32

B = 4
SQ = 4096
SK = 1024
D = 1024
DC = 768
NCORES = 8
SQL = SQ // NCORES  # 512 output rows per batch per core
KC = D // 128       # 8 contraction chunks for q-proj / out-proj
FC = DC // 128      # 6 contraction chunks for k/v-proj
JC = SK // 128      # 8 key chunks
NI = SQ // 512      # 8 query blocks of 512 per batch

Exp = mybir.ActivationFunctionType.Exp
Alu = mybir.AluOpType


def build_nc():
    nc = bacc.Bacc(
        "TRN2",
        target_bir_lowering=False,
        debug=False,
        num_devices=NCORES,
    )

    xt = nc.dram_tensor("xt", [B, KC, 128, SQ], BF16, kind="ExternalInput")
    yt = nc.dram_tensor("yt", [B, FC, 128, SK], BF16, kind="ExternalInput")
    wq = nc.dram_tensor("wq", [KC, 128, 128], BF16, kind="ExternalInput")
    wk = nc.dram_tensor("wk", [FC, 128, 128], BF16, kind="ExternalInput")
    wv = nc.dram_tensor("wv", [FC, 128, 128], BF16, kind="ExternalInput")
    wo = nc.dram_tensor("wo", [KC, 128, D], BF16, kind="ExternalInput")
    bq = nc.dram_tensor("bq", [128, 1], F32, kind="ExternalInput")
    bk = nc.dram_tensor("bk", [128, 1], F32, kind="ExternalInput")
    bvb = nc.dram_tensor("bvb", [128, 128], F32, kind="ExternalInput")
    bob = nc.dram_tensor("bob", [128, D], F32, kind="ExternalInput")
    out = nc.dram_tensor("out", [B, SQL, D], F32, kind="ExternalOutput")

    # DRAM bounce buffers for the per-batch AllToAll.
    send = [
        nc.dram_tensor(f"a2a_send_{b}", [NCORES, 128, 512], BF16, kind="Internal")
        for b in range(B)
    ]
    recv = [
        nc.dram_tensor(f"a2a_recv_{b}", [NCORES, 128, 512], BF16, kind="Internal")
        for b in range(B)
    ]

    with tile.TileContext(nc) as tc:
        _program(nc, tc, xt, yt, wq, wk, wv, wo, bq, bk, bvb, bob, out, send, recv)
    nc.finalize()
    return nc


def _program(nc, tc, xt, yt, wq, wk, wv, wo, bq, bk, bvb, bob, out, send, recv):
    from contextlib import ExitStack

    with ExitStack() as ctx:
        const = ctx.enter_context(tc.tile_pool(name="const", bufs=1))
        ytp = ctx.enter_context(tc.tile_pool(name="ytp", bufs=7))
        xtp = ctx.enter_context(tc.tile_pool(name="xtp", bufs=10))
        qtp = ctx.enter_context(tc.tile_pool(name="qtp", bufs=2))
        ktp = ctx.enter_context(tc.tile_pool(name="ktp", bufs=2))
        vtp = ctx.enter_context(tc.tile_pool(name="vtp", bufs=16))
        ep = ctx.enter_context(tc.tile_pool(name="ep", bufs=3))
        recp = ctx.enter_context(tc.tile_pool(name="recp", bufs=2))
        attp = ctx.enter_context(tc.tile_pool(name="attp", bufs=6))
        rvp = ctx.enter_context(tc.tile_pool(name="rvp", bufs=9))
        outp = ctx.enter_context(tc.tile_pool(name="outp", bufs=2))
        # PSUM: scores 2x2 banks + nout 2x1 + work 2x1 = 8 banks total
        scp = ctx.enter_context(tc.tile_pool(name="scp", bufs=2, space="PSUM"))
        noutp = ctx.enter_context(tc.tile_pool(name="noutp", bufs=2, space="PSUM"))
        workp = ctx.enter_context(tc.tile_pool(name="workp", bufs=2, space="PSUM"))

        # ---- constants / weights resident in SBUF
        bq_sb = const.tile([128, 1], F32, tag="bq")
        nc.sync.dma_start(out=bq_sb[:, :], in_=bq[:, :])
        bk_sb = const.tile([128, 1], F32, tag="bk")
        nc.sync.dma_start(out=bk_sb[:, :], in_=bk[:, :])
        bvb_sb = const.tile([128, 128], F32, tag="bvb")
        nc.sync.dma_start(out=bvb_sb[:, :], in_=bvb[:, :])
        bob_sb = const.tile([128, D], F32, tag="bob")
        nc.sync.dma_start(out=bob_sb[:, :], in_=bob[:, :])
        ones_sb = const.tile([128, 64], BF16, tag="ones")
        nc.vector.memset(ones_sb[:, :], 1.0)

        wq_sb = const.tile([128, KC * 128], BF16, tag="wq")
        for kc in range(KC):
            nc.sync.dma_start(
                out=wq_sb[:, kc * 128:(kc + 1) * 128], in_=wq[kc, :, :]
            )
        wk_sb = const.tile([128, FC * 128], BF16, tag="wk")
        for fc in range(FC):
            nc.sync.dma_start(
                out=wk_sb[:, fc * 128:(fc + 1) * 128], in_=wk[fc, :, :]
            )
        wv_sb = const.tile([128, FC * 128], BF16, tag="wv")
        for fc in range(FC):
            nc.sync.dma_start(
                out=wv_sb[:, fc * 128:(fc + 1) * 128], in_=wv[fc, :, :]
            )
        wo_sb = const.tile([128, KC * D], BF16, tag="wo")
        for kc in range(KC):
            nc.sync.dma_start(
                out=wo_sb[:, kc * D:(kc + 1) * D], in_=wo[kc, :, :]
            )

        v_tiles = {}

        for b in range(B):
            # ---- load yT tiles for this batch
            yts = []
            for fc in range(FC):
                yt_t = ytp.tile([128, SK], BF16, name=f"yt_{b}_{fc}", tag="yt")
                nc.sync.dma_start(out=yt_t[:, :], in_=yt[b, fc, :, :])
                yts.append(yt_t)

            # ---- k projection: kT[c, j] for both heads
            kt_sb = ktp.tile([128, SK], BF16, name=f"kt_{b}", tag="kt")
            for j2 in range(SK // 512):
                kps = workp.tile([128, 512], F32, name=f"kps_{b}_{j2}", tag="work")
                for fc in range(FC):
                    nc.tensor.matmul(
                        kps[:, :],
                        lhsT=wk_sb[:, fc * 128:(fc + 1) * 128],
                        rhs=yts[fc][:, j2 * 512:(j2 + 1) * 512],
                        start=(fc == 0),
                        stop=(fc == FC - 1),
                    )
                nc.vector.tensor_scalar_add(
                    kt_sb[:, j2 * 512:(j2 + 1) * 512], kps[:, :], bk_sb[:, :]
                )

            # ---- v projection, natural layout [j, d], with ones cols appended
            # v_aug layout per tile [128, 130]:
            #   cols 0:64   = head-A values, col 64 = ones (A sums)
            #   cols 65:129 = head-B values, col 129 = ones (B sums)
            for jc in range(JC):
                vps = workp.tile([128, 128], F32, name=f"vps_{b}_{jc}", tag="work")
                for fc in range(FC):
                    nc.tensor.matmul(
                        vps[:, :],
                        lhsT=yts[fc][:, jc * 128:(jc + 1) * 128],
                        rhs=wv_sb[:, fc * 128:(fc + 1) * 128],
                        start=(fc == 0),
                        stop=(fc == FC - 1),
                    )
                v_t = vtp.tile([128, 130], BF16, name=f"v_{b}_{jc}", tag="vt")
                nc.vector.tensor_tensor(
                    out=v_t[:, 0:130].rearrange("p (h x) -> p h x", h=2)[:, :, 0:64],
                    in0=vps[:, :].rearrange("p (h x) -> p h x", h=2),
                    in1=bvb_sb[:, :].rearrange("p (h x) -> p h x", h=2),
                    op=Alu.add,
                )
                nc.vector.memset(v_t[:, 64:65], 1.0)
                nc.vector.memset(v_t[:, 129:130], 1.0)
                v_tiles[(b, jc)] = v_t

            # ---- q projection: qT[c, i], scaled by 1/8, bias folded
            qt_sb = qtp.tile([128, SQ], BF16, name=f"qt_{b}", tag="qt")
            for i5 in range(NI):
                qps = workp.tile([128, 512], F32, name=f"qps_{b}_{i5}", tag="work")
                for kc in range(KC):
                    xt_t = xtp.tile([128, 512], BF16, name=f"xt_{b}_{i5}_{kc}", tag="xt")
                    nc.sync.dma_start(
                        out=xt_t[:, :], in_=xt[b, kc, :, i5 * 512:(i5 + 1) * 512]
                    )
                    nc.tensor.matmul(
                        qps[:, :],
                        lhsT=wq_sb[:, kc * 128:(kc + 1) * 128],
                        rhs=xt_t[:, :],
                        start=(kc == 0),
                        stop=(kc == KC - 1),
                    )
                nc.vector.tensor_scalar(
                    out=qt_sb[:, i5 * 512:(i5 + 1) * 512],
                    in0=qps[:, :],
                    scalar1=bq_sb[:, :],
                    scalar2=0.125,
                    op0=Alu.add,
                    op1=Alu.mult,
                )

            # ---- attention, one 512-wide query block at a time
            for i5 in range(NI):
                isl = slice(i5 * 512, (i5 + 1) * 512)
                na = noutp.tile([65, 512], F32, name=f"na_{b}_{i5}", tag="nout")
                nb = noutp.tile([65, 512], F32, name=f"nb_{b}_{i5}", tag="nout")
                for jc in range(JC):
                    sc = scp.tile([128, 1024], F32, name=f"sc_{b}_{i5}_{jc}", tag="sc")
                    jsl = slice(jc * 128, (jc + 1) * 128)
                    # scoresT for both heads, row-packed (K=64 each)
                    nc.tensor.matmul(
                        sc[:, 0:512],
                        lhsT=kt_sb[0:64, jsl],
                        rhs=qt_sb[0:64, isl],
                        start=True, stop=True,
                    )
                    nc.tensor.matmul(
                        sc[:, 512:1024],
                        lhsT=kt_sb[64:128, jsl],
                        rhs=qt_sb[64:128, isl],
                        start=True, stop=True,
                    )
                    e_t = ep.tile([128, 1024], BF16, name=f"e_{b}_{i5}_{jc}", tag="e")
                    nc.scalar.activation(e_t[:, :], sc[:, :], Exp)
                    v_t = v_tiles[(b, jc)]
                    nc.tensor.matmul(
                        na[:, :],
                        lhsT=v_t[:, 0:65],
                        rhs=e_t[:, 0:512],
                        start=(jc == 0),
                        stop=(jc == JC - 1),
                    )
                    nc.tensor.matmul(
                        nb[:, :],
                        lhsT=v_t[:, 65:130],
                        rhs=e_t[:, 512:1024],
                        start=(jc == 0),
                        stop=(jc == JC - 1),
                    )
                # normalize + emit to the A2A send buffer
                for h, nres in ((0, na), (1, nb)):
                    rec = recp.tile([65, 512], BF16, name=f"rec_{b}_{i5}_{h}", tag="rec")
                    with nc.allow_low_precision(reason="softmax 1/sum in bf16"):
                        nc.vector.reciprocal(rec[64:65, :], nres[64:65, :])
                    bc = workp.tile([128, 512], F32, name=f"bc_{b}_{i5}_{h}", tag="work")
                    nc.tensor.matmul(
                        bc[0:64, :],
                        lhsT=ones_sb[64:65, 0:64],
                        rhs=rec[64:65, :],
                        start=True, stop=True,
                    )
                    bcs = attp.tile([64, 512], BF16, name=f"bcs_{b}_{i5}_{h}", tag="att")
                    nc.vector.tensor_copy(bcs[:, :], bc[0:64, :])
                    att = attp.tile([64, 512], BF16, name=f"att_{b}_{i5}_{h}", tag="att")
                    nc.vector.tensor_mul(att[:, :], nres[0:64, :], bcs[:, :])
                    nc.sync.dma_start(
                        out=send[b][i5, h * 64:(h + 1) * 64, :], in_=att[:, :]
                    )

            # ---- AllToAll for this batch: head-shard -> seq-shard
            nc.gpsimd.collective_compute(
                "AllToAll",
                Alu.bypass,
                replica_groups=[list(range(NCORES))],
                ins=[send[b][:, :, :].opt()],
                outs=[recv[b][:, :, :].opt()],
            )

            # ---- output projection for this batch's 512 rows
            rvs = []
            for cc in range(KC):
                rv = rvp.tile([128, 512], BF16, name=f"rv_{b}_{cc}", tag="rv")
                nc.sync.dma_start(out=rv[:, :], in_=recv[b][cc, :, :])
                rvs.append(rv)
            for i1 in range(SQL // 128):
                for eh in range(2):
                    ops = workp.tile([128, 512], F32, name=f"ops_{b}_{i1}_{eh}", tag="work")
                    for cc in range(KC):
                        nc.tensor.matmul(
                            ops[:, :],
                            lhsT=rvs[cc][:, i1 * 128:(i1 + 1) * 128],
                            rhs=wo_sb[:, cc * D + eh * 512: cc * D + (eh + 1) * 512],
                            start=(cc == 0),
                            stop=(cc == KC - 1),
                        )
                    o_t = outp.tile([128, 512], F32, name=f"o_{b}_{i1}_{eh}", tag="o")
                    nc.vector.tensor_add(
                        o_t[:, :], ops[:, :], bob_sb[:, eh * 512:(eh + 1) * 512]
                    )
                    nc.sync.dma_start(
                        out=out[b, i1 * 128:(i1 + 1) * 128, eh * 512:(eh + 1) * 512],
                        in_=o_t[:, :],
                    )


def prep_in_maps(x, y, Wq, bq, Wk, bk, Wv, bv, Wo, bo):
    bf = ml_dtypes.bfloat16
    x = np.asarray(x, np.float32)
    y = np.asarray(y, np.float32)
    xt = np.ascontiguousarray(x.transpose(0, 2, 1)).reshape(B, KC, 128, SQ).astype(bf)
    yt = np.ascontiguousarray(y.transpose(0, 2, 1)).reshape(B, FC, 128, SK).astype(bf)
    wo = np.ascontiguousarray(np.asarray(Wo, np.float32).reshape(KC, 128, D)).astype(bf)
    bob = np.ascontiguousarray(
        np.broadcast_to(np.asarray(bo, np.float32)[None, :], (128, D))
    )
    in_maps = []
    for c in range(NCORES):
        cs = slice(c * 128, (c + 1) * 128)
        in_maps.append({
            "xt": xt,
            "yt": yt,
            "wq": np.ascontiguousarray(np.asarray(Wq, np.float32)[:, cs].reshape(KC, 128, 128)).astype(bf),
            "wk": np.ascontiguousarray(np.asarray(Wk, np.float32)[:, cs].reshape(FC, 128, 128)).astype(bf),
            "wv": np.ascontiguousarray(np.asarray(Wv, np.float32)[:, cs].reshape(FC, 128, 128)).astype(bf),
            "wo": wo,
            "bq": np.ascontiguousarray(np.asarray(bq, np.float32)[cs].reshape(128, 1)),
            "bk": np.ascontiguousarray(np.asarray(bk, np.float32)[cs].reshape(128, 1)),
            "bvb": np.ascontiguousarray(
                np.broadcast_to(np.asarray(bv, np.float32)[cs][None, :], (128, 128))
            ),
            "bob": bob,
        })
    return in_maps


_NC_CACHE = None


def get_nc():
    global _NC_CACHE
    if _NC_CACHE is None:
        _NC_CACHE = build_nc()
    return _NC_CACHE


def run(in_maps, **kwargs):
    nc = get_nc()
    return bass_utils.run_bass_kernel_spmd(
        nc, in_maps, core_ids=list(range(NCORES)), **kwargs
    )


def gather(results):
    full = np.empty((B, SQ, D), np.float32)
    for c in range(NCORES):
        full[:, c * SQL:(c + 1) * SQL, :] = results[c]["out"]
    return full


def kernel(**inputs):
    in_maps = prep_in_maps(**inputs)
    res = run(in_maps)
    return gather(res.results)


if __name__ == "__main__":
    nc = build_nc()
    print("build OK")


# revision 11
# speedup vs baseline: 1.2460x; 1.2460x over previous
"""Cross-attention (B=4, Sq=4096, Sk=1024, H=16, D=1024) on 8 TRN2 NeuronCores.

Sharding: tensor-parallel by heads. Core c owns heads (2c, 2c+1), i.e. columns
[128c, 128c+128) of Wq/Wk/Wv and rows [128c, 128c+128) of Wo.

Per-core dataflow (all activations kept feature-on-partition, "transposed"):
  qT[c,i] = sum_k Wq[k,c] xT[k,i]        (lhsT=Wq chunk, rhs=xT chunk)
  kT[c,j] likewise from yT; v[j,d] natural layout (lhsT=yT chunk, rhs=Wv chunk)
  scoresT[j,i] = kT_h[:,j].T @ qT_h[:,i]  (per head, row-packed across 2 heads)
  e = exp(scoresT)  (no max-subtraction: scores are O(1) by construction)
  noutT[d,i] (+ sums row via an appended ones column in v_aug) accumulated over j
  attT = noutT * (1/sums) broadcast (PE outer-product broadcast trick)
  AllToAll (per batch) head-shard -> seq-shard; out-proj on 512 rows/batch.

Host prep: x,y,W* are pre-transposed/pre-chunked and cast to bf16 on the host;
all matmuls run bf16 with fp32 PSUM accumulation; output is fp32.
"""

import numpy as np
import ml_dtypes

import concourse.bass as bass
import concourse.mybir as mybir
from concourse import bacc, tile
from concourse import bass_utils

BF16 = mybir.dt.bfloat16
F32 = mybir.dt.float32

B = 4
SQ = 4096
SK = 1024
D = 1024
DC = 768
NCORES = 8
SQL = SQ // NCORES  # 512 output rows per batch per core
KC = D // 128       # 8 contraction chunks for q-proj / out-proj
FC = DC // 128      # 6 contraction chunks for k/v-proj
JC = SK // 128      # 8 key chunks
NI = SQ // 512      # 8 query blocks of 512 per batch

Exp = mybir.ActivationFunctionType.Exp
Alu = mybir.AluOpType


def build_nc():
    nc = bacc.Bacc(
        "TRN2",
        target_bir_lowering=False,
        debug=False,
        num_devices=NCORES,
    )

    xt = nc.dram_tensor("xt", [B, KC, 128, SQ], BF16, kind="ExternalInput")
    yt = nc.dram_tensor("yt", [B, FC, 128, SK], BF16, kind="ExternalInput")
    wq = nc.dram_tensor("wq", [KC, 128, 128], BF16, kind="ExternalInput")
    wk = nc.dram_tensor("wk", [FC, 128, 128], BF16, kind="ExternalInput")
    wv = nc.dram_tensor("wv", [FC, 128, 128], BF16, kind="ExternalInput")
    wo = nc.dram_tensor("wo", [KC, 128, D], BF16, kind="ExternalInput")
    bq = nc.dram_tensor("bq", [128, 1], F32, kind="ExternalInput")
    bk = nc.dram_tensor("bk", [128, 1], F32, kind="ExternalInput")
    bvb = nc.dram_tensor("bvb", [128, 128], F32, kind="ExternalInput")
    bob = nc.dram_tensor("bob", [128, D], F32, kind="ExternalInput")
    out = nc.dram_tensor("out", [B, SQL, D], F32, kind="ExternalOutput")

    # DRAM bounce buffers for the per-batch AllToAll.
    send = [
        nc.dram_tensor(f"a2a_send_{b}", [NCORES, 128, 512], BF16, kind="Internal")
        for b in range(B)
    ]
    recv = [
        nc.dram_tensor(f"a2a_recv_{b}", [NCORES, 128, 512], BF16, kind="Internal")
        for b in range(B)
    ]

    with tile.TileContext(nc) as tc:
        _program(nc, tc, xt, yt, wq, wk, wv, wo, bq, bk, bvb, bob, out, send, recv)
    nc.finalize()
    return nc


def _program(nc, tc, xt, yt, wq, wk, wv, wo, bq, bk, bvb, bob, out, send, recv):
    from contextlib import ExitStack

    with ExitStack() as ctx:
        const = ctx.enter_context(tc.tile_pool(name="const", bufs=1))
        ytp = ctx.enter_context(tc.tile_pool(name="ytp", bufs=7))
        xtp = ctx.enter_context(tc.tile_pool(name="xtp", bufs=10))
        bcp = ctx.enter_context(tc.tile_pool(name="bcp", bufs=4))
        qtp = ctx.enter_context(tc.tile_pool(name="qtp", bufs=2))
        ktp = ctx.enter_context(tc.tile_pool(name="ktp", bufs=2))
        vtp = ctx.enter_context(tc.tile_pool(name="vtp", bufs=16))
        ep = ctx.enter_context(tc.tile_pool(name="ep", bufs=3))
        recp = ctx.enter_context(tc.tile_pool(name="recp", bufs=2))
        attp = ctx.enter_context(tc.tile_pool(name="attp", bufs=6))
        rvp = ctx.enter_context(tc.tile_pool(name="rvp", bufs=9))
        outp = ctx.enter_context(tc.tile_pool(name="outp", bufs=2))
        # PSUM: scores 2x2 banks + nout 2x1 + proj 1 + outproj 1 = 8 banks
        scp = ctx.enter_context(tc.tile_pool(name="scp", bufs=2, space="PSUM"))
        noutp = ctx.enter_context(tc.tile_pool(name="noutp", bufs=2, space="PSUM"))
        projp = ctx.enter_context(tc.tile_pool(name="projp", bufs=1, space="PSUM"))
        outpp = ctx.enter_context(tc.tile_pool(name="outpp", bufs=1, space="PSUM"))
        rbp = ctx.enter_context(tc.tile_pool(name="rbp", bufs=4, space="DRAM"))

        # ---- constants / weights resident in SBUF
        bq_sb = const.tile([128, 1], F32, tag="bq")
        nc.sync.dma_start(out=bq_sb[:, :], in_=bq[:, :])
        bk_sb = const.tile([128, 1], F32, tag="bk")
        nc.sync.dma_start(out=bk_sb[:, :], in_=bk[:, :])
        bvb_sb = const.tile([128, 128], F32, tag="bvb")
        nc.sync.dma_start(out=bvb_sb[:, :], in_=bvb[:, :])
        bob_sb = const.tile([128, D], F32, tag="bob")
        nc.sync.dma_start(out=bob_sb[:, :], in_=bob[:, :])

        wq_sb = const.tile([128, KC * 128], BF16, tag="wq")
        for kc in range(KC):
            nc.sync.dma_start(
                out=wq_sb[:, kc * 128:(kc + 1) * 128], in_=wq[kc, :, :]
            )
        wk_sb = const.tile([128, FC * 128], BF16, tag="wk")
        for fc in range(FC):
            nc.sync.dma_start(
                out=wk_sb[:, fc * 128:(fc + 1) * 128], in_=wk[fc, :, :]
            )
        wv_sb = const.tile([128, FC * 128], BF16, tag="wv")
        for fc in range(FC):
            nc.sync.dma_start(
                out=wv_sb[:, fc * 128:(fc + 1) * 128], in_=wv[fc, :, :]
            )
        wo_sb = const.tile([128, KC * D], BF16, tag="wo")
        for kc in range(KC):
            nc.sync.dma_start(
                out=wo_sb[:, kc * D:(kc + 1) * D], in_=wo[kc, :, :]
            )

        v_tiles = {}
        rv_tiles = {}

        def emit_outproj_chunk(ob, chunk):
            # output projection for batch ob, one (i1, eh) chunk of 8
            i1, eh = divmod(chunk, 2)
            rvs = rv_tiles[ob]
            ops = outpp.tile(
                [128, 512], F32, name=f"ops_{ob}_{i1}_{eh}", tag="ops"
            )
            for cc in range(KC):
                nc.tensor.matmul(
                    ops[:, :],
                    lhsT=rvs[cc][:, i1 * 128:(i1 + 1) * 128],
                    rhs=wo_sb[:, cc * D + eh * 512: cc * D + (eh + 1) * 512],
                    start=(cc == 0),
                    stop=(cc == KC - 1),
                )
            o_t = outp.tile([128, 512], F32, name=f"o_{ob}_{i1}_{eh}", tag="o")
            nc.vector.tensor_add(
                o_t[:, :], ops[:, :], bob_sb[:, eh * 512:(eh + 1) * 512]
            )
            nc.sync.dma_start(
                out=out[ob, i1 * 128:(i1 + 1) * 128, eh * 512:(eh + 1) * 512],
                in_=o_t[:, :],
            )

        for b in range(B):
            # ---- load yT tiles for this batch
            yts = []
            for fc in range(FC):
                yt_t = ytp.tile([128, SK], BF16, name=f"yt_{b}_{fc}", tag="yt")
                nc.sync.dma_start(out=yt_t[:, :], in_=yt[b, fc, :, :])
                yts.append(yt_t)

            # ---- k projection: kT[c, j] for both heads
            kt_sb = ktp.tile([128, SK], BF16, name=f"kt_{b}", tag="kt")
            for j2 in range(SK // 512):
                kps = projp.tile([128, 512], F32, name=f"kps_{b}_{j2}", tag="proj")
                for fc in range(FC):
                    nc.tensor.matmul(
                        kps[:, :],
                        lhsT=wk_sb[:, fc * 128:(fc + 1) * 128],
                        rhs=yts[fc][:, j2 * 512:(j2 + 1) * 512],
                        start=(fc == 0),
                        stop=(fc == FC - 1),
                    )
                nc.vector.tensor_scalar_add(
                    kt_sb[:, j2 * 512:(j2 + 1) * 512], kps[:, :], bk_sb[:, :]
                )

            # ---- v projection, natural layout [j, d], with ones cols appended
            # v_aug layout per tile [128, 130]:
            #   cols 0:64   = head-A values, col 64 = ones (A sums)
            #   cols 65:129 = head-B values, col 129 = ones (B sums)
            for jc in range(JC):
                vps = projp.tile([128, 128], F32, name=f"vps_{b}_{jc}", tag="proj")
                for fc in range(FC):
                    nc.tensor.matmul(
                        vps[:, :],
                        lhsT=yts[fc][:, jc * 128:(jc + 1) * 128],
                        rhs=wv_sb[:, fc * 128:(fc + 1) * 128],
                        start=(fc == 0),
                        stop=(fc == FC - 1),
                    )
                v_t = vtp.tile([128, 130], BF16, name=f"v_{b}_{jc}", tag="vt")
                nc.vector.tensor_tensor(
                    out=v_t[:, 0:130].rearrange("p (h x) -> p h x", h=2)[:, :, 0:64],
                    in0=vps[:, :].rearrange("p (h x) -> p h x", h=2),
                    in1=bvb_sb[:, :].rearrange("p (h x) -> p h x", h=2),
                    op=Alu.add,
                )
                nc.vector.memset(v_t[:, 64:65], 1.0)
                nc.vector.memset(v_t[:, 129:130], 1.0)
                v_tiles[(b, jc)] = v_t

            # ---- q projection: qT[c, i], scaled by 1/8, bias folded
            qt_sb = qtp.tile([128, SQ], BF16, name=f"qt_{b}", tag="qt")
            xt_tiles = {}
            for i5 in range(NI):
                if i5 % 2 == 0:
                    g = i5 // 2
                    for kc in range(KC):
                        t = xtp.tile([128, 1024], BF16, name=f"xt_{b}_{g}_{kc}", tag="xt")
                        nc.sync.dma_start(
                            out=t[:, :], in_=xt[b, kc, :, g * 1024:(g + 1) * 1024]
                        )
                        xt_tiles[kc] = t
                qps = projp.tile([128, 512], F32, name=f"qps_{b}_{i5}", tag="proj")
                for kc in range(KC):
                    nc.tensor.matmul(
                        qps[:, :],
                        lhsT=wq_sb[:, kc * 128:(kc + 1) * 128],
                        rhs=xt_tiles[kc][:, (i5 % 2) * 512:(i5 % 2) * 512 + 512],
                        start=(kc == 0),
                        stop=(kc == KC - 1),
                    )
                nc.vector.tensor_scalar(
                    out=qt_sb[:, i5 * 512:(i5 + 1) * 512],
                    in0=qps[:, :],
                    scalar1=bq_sb[:, :],
                    scalar2=0.125,
                    op0=Alu.add,
                    op1=Alu.mult,
                )

            # ---- attention, one 512-wide query block at a time
            for i5 in range(NI):
                isl = slice(i5 * 512, (i5 + 1) * 512)
                na = noutp.tile([65, 512], F32, name=f"na_{b}_{i5}", tag="nout")
                nb = noutp.tile([65, 512], F32, name=f"nb_{b}_{i5}", tag="nout")
                for jc in range(JC):
                    sc = scp.tile([128, 1024], F32, name=f"sc_{b}_{i5}_{jc}", tag="sc")
                    jsl = slice(jc * 128, (jc + 1) * 128)
                    # scoresT for both heads, row-packed (K=64 each)
                    nc.tensor.matmul(
                        sc[:, 0:512],
                        lhsT=kt_sb[0:64, jsl],
                        rhs=qt_sb[0:64, isl],
                        start=True, stop=True,
                    )
                    nc.tensor.matmul(
                        sc[:, 512:1024],
                        lhsT=kt_sb[64:128, jsl],
                        rhs=qt_sb[64:128, isl],
                        start=True, stop=True,
                    )
                    e_t = ep.tile([128, 1024], BF16, name=f"e_{b}_{i5}_{jc}", tag="e")
                    nc.scalar.activation(e_t[:, :], sc[:, :], Exp)
                    v_t = v_tiles[(b, jc)]
                    nc.tensor.matmul(
                        na[:, :],
                        lhsT=v_t[:, 0:65],
                        rhs=e_t[:, 0:512],
                        start=(jc == 0),
                        stop=(jc == JC - 1),
                    )
                    nc.tensor.matmul(
                        nb[:, :],
                        lhsT=v_t[:, 65:130],
                        rhs=e_t[:, 512:1024],
                        start=(jc == 0),
                        stop=(jc == JC - 1),
                    )
                # normalize + emit to the A2A send buffer
                for h, nres in ((0, na), (1, nb)):
                    rec = recp.tile([65, 512], F32, name=f"rec_{b}_{i5}_{h}", tag="rec")
                    # NB: base_partition must be 0 for the custom DVE op
                    # (row-64-only slices produce garbage on HW), so compute
                    # 1/x over the whole tile and use just the sums row.
                    nc.vector.reciprocal_approx_fast(
                        out=rec[:, :], in_=nres[:, :]
                    )
                    rb = rbp.tile([1, 512], F32, name=f"rb_{b}_{i5}_{h}", tag="rb")
                    nc.gpsimd.dma_start(out=rb[:, :], in_=rec[64:65, :])
                    bc = bcp.tile([64, 512], F32, name=f"bc_{b}_{i5}_{h}", tag="bc")
                    nc.gpsimd.dma_start(
                        out=bc[:, :], in_=rb[0:1, :].to_broadcast([64, 512])
                    )
                    att = attp.tile([64, 512], BF16, name=f"att_{b}_{i5}_{h}", tag="att")
                    nc.vector.tensor_mul(att[:, :], nres[0:64, :], bc[:, :])
                    nc.gpsimd.dma_start(
                        out=send[b][i5, h * 64:(h + 1) * 64, :], in_=att[:, :]
                    )
                if b > 0:
                    emit_outproj_chunk(b - 1, i5)

            # ---- AllToAll for this batch: head-shard -> seq-shard
            nc.gpsimd.collective_compute(
                "AllToAll",
                Alu.bypass,
                replica_groups=[list(range(NCORES))],
                ins=[send[b][:, :, :].opt()],
                outs=[recv[b][:, :, :].opt()],
            )
            rvs = []
            for cc in range(KC):
                rv = rvp.tile([128, 512], BF16, name=f"rv_{b}_{cc}", tag="rv")
                nc.sync.dma_start(out=rv[:, :], in_=recv[b][cc, :, :])
                rvs.append(rv)
            rv_tiles[b] = rvs

        # drain the last batch's output projection
        for chunk in range(8):
            emit_outproj_chunk(B - 1, chunk)


def prep_in_maps(x, y, Wq, bq, Wk, bk, Wv, bv, Wo, bo):
    bf = ml_dtypes.bfloat16
    x = np.asarray(x, np.float32)
    y = np.asarray(y, np.float32)
    xt = np.ascontiguousarray(x.transpose(0, 2, 1)).reshape(B, KC, 128, SQ).astype(bf)
    yt = np.ascontiguousarray(y.transpose(0, 2, 1)).reshape(B, FC, 128, SK).astype(bf)
    wo = np.ascontiguousarray(np.asarray(Wo, np.float32).reshape(KC, 128, D)).astype(bf)
    bob = np.ascontiguousarray(
        np.broadcast_to(np.asarray(bo, np.float32)[None, :], (128, D))
    )
    in_maps = []
    for c in range(NCORES):
        cs = slice(c * 128, (c + 1) * 128)
        in_maps.append({
            "xt": xt,
            "yt": yt,
            "wq": np.ascontiguousarray(np.asarray(Wq, np.float32)[:, cs].reshape(KC, 128, 128)).astype(bf),
            "wk": np.ascontiguousarray(np.asarray(Wk, np.float32)[:, cs].reshape(FC, 128, 128)).astype(bf),
            "wv": np.ascontiguousarray(np.asarray(Wv, np.float32)[:, cs].reshape(FC, 128, 128)).astype(bf),
            "wo": wo,
            "bq": np.ascontiguousarray(np.asarray(bq, np.float32)[cs].reshape(128, 1)),
            "bk": np.ascontiguousarray(np.asarray(bk, np.float32)[cs].reshape(128, 1)),
            "bvb": np.ascontiguousarray(
                np.broadcast_to(np.asarray(bv, np.float32)[cs][None, :], (128, 128))
            ),
            "bob": bob,
        })
    return in_maps


_NC_CACHE = None


def get_nc():
    global _NC_CACHE
    if _NC_CACHE is None:
        _NC_CACHE = build_nc()
    return _NC_CACHE


def run(in_maps, **kwargs):
    nc = get_nc()
    return bass_utils.run_bass_kernel_spmd(
        nc, in_maps, core_ids=list(range(NCORES)), **kwargs
    )


def gather(results):
    full = np.empty((B, SQ, D), np.float32)
    for c in range(NCORES):
        full[:, c * SQL:(c + 1) * SQL, :] = results[c]["out"]
    return full


def kernel(**inputs):
    in_maps = prep_in_maps(**inputs)
    res = run(in_maps)
    return gather(res.results)


if __name__ == "__main__":
    nc = build_nc()
    print("build OK")


# revision 13
# speedup vs baseline: 1.4700x; 1.1798x over previous
"""Cross-attention (B=4, Sq=4096, Sk=1024, H=16, D=1024) on 8 TRN2 NeuronCores.

Sharding: tensor-parallel by heads. Core c owns heads (2c, 2c+1), i.e. columns
[128c, 128c+128) of Wq/Wk/Wv and rows [128c, 128c+128) of Wo.

Per-core dataflow (all activations kept feature-on-partition, "transposed"):
  qT[c,i] = sum_k Wq[k,c] xT[k,i]        (lhsT=Wq chunk, rhs=xT chunk)
  kT[c,j] likewise from yT; v[j,d] natural layout (lhsT=yT chunk, rhs=Wv chunk)
  scoresT[j,i] = kT_h[:,j].T @ qT_h[:,i]  (per head, row-packed across 2 heads)
  e = exp(scoresT)  (no max-subtraction: scores are O(1) by construction)
  noutT[d,i] (+ sums row via an appended ones column in v_aug) accumulated over j
  attT = noutT * (1/sums) broadcast (PE outer-product broadcast trick)
  AllToAll (per batch) head-shard -> seq-shard; out-proj on 512 rows/batch.

Host prep: x,y,W* are pre-transposed/pre-chunked and cast to bf16 on the host;
all matmuls run bf16 with fp32 PSUM accumulation; output is fp32.
"""

import numpy as np
import ml_dtypes

import concourse.bass as bass
import concourse.mybir as mybir
from concourse import bacc, tile
from concourse import bass_utils

BF16 = mybir.dt.bfloat16
F32 = mybir.dt.float32

B = 4
SQ = 4096
SK = 1024
D = 1024
DC = 768
NCORES = 8
SQL = SQ // NCORES  # 512 output rows per batch per core
KC = D // 128       # 8 contraction chunks for q-proj / out-proj
FC = DC // 128      # 6 contraction chunks for k/v-proj
JC = SK // 128      # 8 key chunks
NI = SQ // 512      # 8 query blocks of 512 per batch

Exp = mybir.ActivationFunctionType.Exp
Alu = mybir.AluOpType


def build_nc():
    nc = bacc.Bacc(
        "TRN2",
        target_bir_lowering=False,
        debug=False,
        num_devices=NCORES,
    )

    xt = nc.dram_tensor("xt", [B, KC, 128, SQ], BF16, kind="ExternalInput")
    yt = nc.dram_tensor("yt", [B, FC, 128, SK], BF16, kind="ExternalInput")
    wq = nc.dram_tensor("wq", [KC, 128, 128], BF16, kind="ExternalInput")
    wk = nc.dram_tensor("wk", [FC, 128, 128], BF16, kind="ExternalInput")
    wv = nc.dram_tensor("wv", [FC, 128, 128], BF16, kind="ExternalInput")
    wo = nc.dram_tensor("wo", [KC, 128, D], BF16, kind="ExternalInput")
    bq = nc.dram_tensor("bq", [128, 1], F32, kind="ExternalInput")
    bk = nc.dram_tensor("bk", [128, 1], F32, kind="ExternalInput")
    bvb = nc.dram_tensor("bvb", [128, 128], F32, kind="ExternalInput")
    bob = nc.dram_tensor("bob", [128, D], F32, kind="ExternalInput")
    out = nc.dram_tensor("out", [B, SQL, D], F32, kind="ExternalOutput")

    # DRAM bounce buffers for the per-batch AllToAll.
    send = [
        nc.dram_tensor(f"a2a_send_{b}", [NCORES, 128, 512], BF16, kind="Internal")
        for b in range(B)
    ]
    recv = [
        nc.dram_tensor(f"a2a_recv_{b}", [NCORES, 128, 512], BF16, kind="Internal")
        for b in range(B)
    ]

    with tile.TileContext(nc) as tc:
        _program(nc, tc, xt, yt, wq, wk, wv, wo, bq, bk, bvb, bob, out, send, recv)
    nc.finalize()
    return nc


def _program(nc, tc, xt, yt, wq, wk, wv, wo, bq, bk, bvb, bob, out, send, recv):
    from contextlib import ExitStack

    with ExitStack() as ctx:
        const = ctx.enter_context(tc.tile_pool(name="const", bufs=1))
        ytp = ctx.enter_context(tc.tile_pool(name="ytp", bufs=7))
        xtp = ctx.enter_context(tc.tile_pool(name="xtp", bufs=10))
        bcp = ctx.enter_context(tc.tile_pool(name="bcp", bufs=4))
        nrp = ctx.enter_context(tc.tile_pool(name="nrp", bufs=4))
        qtp = ctx.enter_context(tc.tile_pool(name="qtp", bufs=2))
        ktp = ctx.enter_context(tc.tile_pool(name="ktp", bufs=2))
        vtp = ctx.enter_context(tc.tile_pool(name="vtp", bufs=16))
        ep = ctx.enter_context(tc.tile_pool(name="ep", bufs=3))
        recp = ctx.enter_context(tc.tile_pool(name="recp", bufs=2))
        attp = ctx.enter_context(tc.tile_pool(name="attp", bufs=6))
        rvp = ctx.enter_context(tc.tile_pool(name="rvp", bufs=9))
        outp = ctx.enter_context(tc.tile_pool(name="outp", bufs=2))
        # PSUM: scores 2x2 banks + nout 2x1 + proj 1 + outproj 1 = 8 banks
        scp = ctx.enter_context(tc.tile_pool(name="scp", bufs=2, space="PSUM"))
        noutp = ctx.enter_context(tc.tile_pool(name="noutp", bufs=2, space="PSUM"))
        projp = ctx.enter_context(tc.tile_pool(name="projp", bufs=1, space="PSUM"))
        outpp = ctx.enter_context(tc.tile_pool(name="outpp", bufs=1, space="PSUM"))
        rbp = ctx.enter_context(tc.tile_pool(name="rbp", bufs=4, space="DRAM"))

        # ---- constants / weights resident in SBUF
        bq_sb = const.tile([128, 1], F32, tag="bq")
        nc.sync.dma_start(out=bq_sb[:, :], in_=bq[:, :])
        bk_sb = const.tile([128, 1], F32, tag="bk")
        nc.sync.dma_start(out=bk_sb[:, :], in_=bk[:, :])
        bvb_sb = const.tile([128, 128], F32, tag="bvb")
        nc.sync.dma_start(out=bvb_sb[:, :], in_=bvb[:, :])
        bob_sb = const.tile([128, D], F32, tag="bob")
        nc.sync.dma_start(out=bob_sb[:, :], in_=bob[:, :])

        wq_sb = const.tile([128, KC * 128], BF16, tag="wq")
        for kc in range(KC):
            nc.sync.dma_start(
                out=wq_sb[:, kc * 128:(kc + 1) * 128], in_=wq[kc, :, :]
            )
        wk_sb = const.tile([128, FC * 128], BF16, tag="wk")
        for fc in range(FC):
            nc.sync.dma_start(
                out=wk_sb[:, fc * 128:(fc + 1) * 128], in_=wk[fc, :, :]
            )
        wv_sb = const.tile([128, FC * 128], BF16, tag="wv")
        for fc in range(FC):
            nc.sync.dma_start(
                out=wv_sb[:, fc * 128:(fc + 1) * 128], in_=wv[fc, :, :]
            )
        wo_sb = const.tile([128, KC * D], BF16, tag="wo")
        for kc in range(KC):
            nc.sync.dma_start(
                out=wo_sb[:, kc * D:(kc + 1) * D], in_=wo[kc, :, :]
            )

        v_tiles = {}
        rv_tiles = {}

        def emit_outproj_chunk(ob, chunk, pool=None, tag="ops"):
            # output projection for batch ob, one (i1, eh) chunk of 8
            i1, eh = divmod(chunk, 2)
            rvs = rv_tiles[ob]
            ops = (pool or outpp).tile(
                [128, 512], F32, name=f"ops_{ob}_{i1}_{eh}", tag=tag
            )
            for cc in range(KC):
                nc.tensor.matmul(
                    ops[:, :],
                    lhsT=rvs[cc][:, i1 * 128:(i1 + 1) * 128],
                    rhs=wo_sb[:, cc * D + eh * 512: cc * D + (eh + 1) * 512],
                    start=(cc == 0),
                    stop=(cc == KC - 1),
                )
            o_t = outp.tile([128, 512], F32, name=f"o_{ob}_{i1}_{eh}", tag="o")
            nc.vector.tensor_add(
                o_t[:, :], ops[:, :], bob_sb[:, eh * 512:(eh + 1) * 512]
            )
            nc.sync.dma_start(
                out=out[ob, i1 * 128:(i1 + 1) * 128, eh * 512:(eh + 1) * 512],
                in_=o_t[:, :],
            )

        for b in range(B):
            # ---- load yT tiles for this batch
            yts = []
            for fc in range(FC):
                yt_t = ytp.tile([128, SK], BF16, name=f"yt_{b}_{fc}", tag="yt")
                nc.sync.dma_start(out=yt_t[:, :], in_=yt[b, fc, :, :])
                yts.append(yt_t)

            # ---- k projection: kT[c, j] for both heads
            kt_sb = ktp.tile([128, SK], BF16, name=f"kt_{b}", tag="kt")
            for j2 in range(SK // 512):
                kps = projp.tile([128, 512], F32, name=f"kps_{b}_{j2}", tag="proj")
                for fc in range(FC):
                    nc.tensor.matmul(
                        kps[:, :],
                        lhsT=wk_sb[:, fc * 128:(fc + 1) * 128],
                        rhs=yts[fc][:, j2 * 512:(j2 + 1) * 512],
                        start=(fc == 0),
                        stop=(fc == FC - 1),
                    )
                nc.vector.tensor_scalar_add(
                    kt_sb[:, j2 * 512:(j2 + 1) * 512], kps[:, :], bk_sb[:, :]
                )

            # ---- v projection, natural layout [j, d], with ones cols appended
            # v_aug layout per tile [128, 130]:
            #   cols 0:64   = head-A values, col 64 = ones (A sums)
            #   cols 65:129 = head-B values, col 129 = ones (B sums)
            for jc in range(JC):
                vps = projp.tile([128, 128], F32, name=f"vps_{b}_{jc}", tag="proj")
                for fc in range(FC):
                    nc.tensor.matmul(
                        vps[:, :],
                        lhsT=yts[fc][:, jc * 128:(jc + 1) * 128],
                        rhs=wv_sb[:, fc * 128:(fc + 1) * 128],
                        start=(fc == 0),
                        stop=(fc == FC - 1),
                    )
                v_t = vtp.tile([128, 130], BF16, name=f"v_{b}_{jc}", tag="vt")
                nc.vector.tensor_tensor(
                    out=v_t[:, 0:130].rearrange("p (h x) -> p h x", h=2)[:, :, 0:64],
                    in0=vps[:, :].rearrange("p (h x) -> p h x", h=2),
                    in1=bvb_sb[:, :].rearrange("p (h x) -> p h x", h=2),
                    op=Alu.add,
                )
                nc.vector.memset(v_t[:, 64:65], 1.0)
                nc.vector.memset(v_t[:, 129:130], 1.0)
                v_tiles[(b, jc)] = v_t

            # ---- q projection: qT[c, i], scaled by 1/8, bias folded
            qt_sb = qtp.tile([128, SQ], BF16, name=f"qt_{b}", tag="qt")
            xt_tiles = {}
            for i5 in range(NI):
                if i5 % 2 == 0:
                    g = i5 // 2
                    for kc in range(KC):
                        t = xtp.tile([128, 1024], BF16, name=f"xt_{b}_{g}_{kc}", tag="xt")
                        nc.sync.dma_start(
                            out=t[:, :], in_=xt[b, kc, :, g * 1024:(g + 1) * 1024]
                        )
                        xt_tiles[kc] = t
                qps = projp.tile([128, 512], F32, name=f"qps_{b}_{i5}", tag="proj")
                for kc in range(KC):
                    nc.tensor.matmul(
                        qps[:, :],
                        lhsT=wq_sb[:, kc * 128:(kc + 1) * 128],
                        rhs=xt_tiles[kc][:, (i5 % 2) * 512:(i5 % 2) * 512 + 512],
                        start=(kc == 0),
                        stop=(kc == KC - 1),
                    )
                nc.vector.tensor_scalar(
                    out=qt_sb[:, i5 * 512:(i5 + 1) * 512],
                    in0=qps[:, :],
                    scalar1=bq_sb[:, :],
                    scalar2=0.125,
                    op0=Alu.add,
                    op1=Alu.mult,
                )

            # ---- attention, one 512-wide query block at a time
            for i5 in range(NI):
                isl = slice(i5 * 512, (i5 + 1) * 512)
                na = noutp.tile([65, 512], F32, name=f"na_{b}_{i5}", tag="nout")
                nb = noutp.tile([65, 512], F32, name=f"nb_{b}_{i5}", tag="nout")
                for jc in range(JC):
                    sc = scp.tile([128, 1024], F32, name=f"sc_{b}_{i5}_{jc}", tag="sc")
                    jsl = slice(jc * 128, (jc + 1) * 128)
                    # scoresT for both heads, row-packed (K=64 each)
                    nc.tensor.matmul(
                        sc[:, 0:512],
                        lhsT=kt_sb[0:64, jsl],
                        rhs=qt_sb[0:64, isl],
                        start=True, stop=True,
                    )
                    nc.tensor.matmul(
                        sc[:, 512:1024],
                        lhsT=kt_sb[64:128, jsl],
                        rhs=qt_sb[64:128, isl],
                        start=True, stop=True,
                    )
                    e_t = ep.tile([128, 1024], BF16, name=f"e_{b}_{i5}_{jc}", tag="e")
                    nc.scalar.activation(e_t[:, :], sc[:, :], Exp)
                    v_t = v_tiles[(b, jc)]
                    nc.tensor.matmul(
                        na[:, :],
                        lhsT=v_t[:, 0:65],
                        rhs=e_t[:, 0:512],
                        start=(jc == 0),
                        stop=(jc == JC - 1),
                    )
                    nc.tensor.matmul(
                        nb[:, :],
                        lhsT=v_t[:, 65:130],
                        rhs=e_t[:, 512:1024],
                        start=(jc == 0),
                        stop=(jc == JC - 1),
                    )
                # normalize + emit to the A2A send buffer
                for h, nres in ((0, na), (1, nb)):
                    # evacuate the psum bank ASAP so the next block's nout
                    # matmuls can start; everything below reads SBUF
                    nsb = nrp.tile([65, 512], F32, name=f"nsb_{b}_{i5}_{h}", tag="nr")
                    nc.vector.tensor_copy(nsb[:, :], nres[:, :])
                    rec = recp.tile([65, 512], F32, name=f"rec_{b}_{i5}_{h}", tag="rec")
                    # NB: base_partition must be 0 for the custom DVE op
                    # (row-64-only slices produce garbage on HW), so compute
                    # 1/x over the whole tile and use just the sums row.
                    nc.vector.reciprocal_approx_fast(
                        out=rec[:, :], in_=nsb[:, :]
                    )
                    rb = rbp.tile([1, 512], F32, name=f"rb_{b}_{i5}_{h}", tag="rb")
                    nc.gpsimd.dma_start(out=rb[:, :], in_=rec[64:65, :])
                    bc = bcp.tile([64, 512], F32, name=f"bc_{b}_{i5}_{h}", tag="bc")
                    nc.gpsimd.dma_start(
                        out=bc[:, :], in_=rb[0:1, :].to_broadcast([64, 512])
                    )
                    att = attp.tile([64, 512], BF16, name=f"att_{b}_{i5}_{h}", tag="att")
                    nc.vector.tensor_mul(att[:, :], nsb[0:64, :], bc[:, :])
                    nc.gpsimd.dma_start(
                        out=send[b][i5, h * 64:(h + 1) * 64, :], in_=att[:, :]
                    )
                if b > 0:
                    emit_outproj_chunk(b - 1, i5)

            # ---- AllToAll for this batch: head-shard -> seq-shard
            nc.gpsimd.collective_compute(
                "AllToAll",
                Alu.bypass,
                replica_groups=[list(range(NCORES))],
                ins=[send[b][:, :, :].opt()],
                outs=[recv[b][:, :, :].opt()],
            )
            rvs = []
            for cc in range(KC):
                rv = rvp.tile([128, 512], BF16, name=f"rv_{b}_{cc}", tag="rv")
                nc.gpsimd.dma_start(out=rv[:, :], in_=recv[b][cc, :, :])
                rvs.append(rv)
            rv_tiles[b] = rvs

        # drain the last batch's output projection, rotating over the
        # now-idle psum pools for pipelining depth
        for chunk in range(8):
            pool, tag = [(outpp, "ops"), (scp, "sc"),
                         (noutp, "nout"), (scp, "sc")][chunk % 4]
            emit_outproj_chunk(B - 1, chunk, pool=pool, tag=tag)


def prep_in_maps(x, y, Wq, bq, Wk, bk, Wv, bv, Wo, bo):
    bf = ml_dtypes.bfloat16
    x = np.asarray(x, np.float32)
    y = np.asarray(y, np.float32)
    xt = np.ascontiguousarray(x.transpose(0, 2, 1)).reshape(B, KC, 128, SQ).astype(bf)
    yt = np.ascontiguousarray(y.transpose(0, 2, 1)).reshape(B, FC, 128, SK).astype(bf)
    wo = np.ascontiguousarray(np.asarray(Wo, np.float32).reshape(KC, 128, D)).astype(bf)
    bob = np.ascontiguousarray(
        np.broadcast_to(np.asarray(bo, np.float32)[None, :], (128, D))
    )
    in_maps = []
    for c in range(NCORES):
        cs = slice(c * 128, (c + 1) * 128)
        in_maps.append({
            "xt": xt,
            "yt": yt,
            "wq": np.ascontiguousarray(np.asarray(Wq, np.float32)[:, cs].reshape(KC, 128, 128)).astype(bf),
            "wk": np.ascontiguousarray(np.asarray(Wk, np.float32)[:, cs].reshape(FC, 128, 128)).astype(bf),
            "wv": np.ascontiguousarray(np.asarray(Wv, np.float32)[:, cs].reshape(FC, 128, 128)).astype(bf),
            "wo": wo,
            "bq": np.ascontiguousarray(np.asarray(bq, np.float32)[cs].reshape(128, 1)),
            "bk": np.ascontiguousarray(np.asarray(bk, np.float32)[cs].reshape(128, 1)),
            "bvb": np.ascontiguousarray(
                np.broadcast_to(np.asarray(bv, np.float32)[cs][None, :], (128, 128))
            ),
            "bob": bob,
        })
    return in_maps


_NC_CACHE = None


def get_nc():
    global _NC_CACHE
    if _NC_CACHE is None:
        _NC_CACHE = build_nc()
    return _NC_CACHE


def run(in_maps, **kwargs):
    nc = get_nc()
    return bass_utils.run_bass_kernel_spmd(
        nc, in_maps, core_ids=list(range(NCORES)), **kwargs
    )


def gather(results):
    full = np.empty((B, SQ, D), np.float32)
    for c in range(NCORES):
        full[:, c * SQL:(c + 1) * SQL, :] = results[c]["out"]
    return full


def kernel(**inputs):
    in_maps = prep_in_maps(**inputs)
    res = run(in_maps)
    return gather(res.results)


if __name__ == "__main__":
    nc = build_nc()
    print("build OK")
